# revision 12
# baseline (speedup 1.0000x reference)
"""DepthToSpace (DCR, block=2) Trainium2 Bass kernel.

Full op: input [16, 256, 128, 128] f32 -> output [16, 64, 256, 256] f32
  out[b, c, 2h+j, 2w+r] = in[b, (2j+r)*64 + c, h, w]

Sharding: batch dim split across 8 NeuronCores (2 batches/core), no
communication.

Per-core design (HW-measured on the axon-tunneled TRN2 pool):
  - SBUF partition dim = (b, c) [2*64 = 128 partitions]; tile over H in
    blocks of HT=16 rows.  Every DMA descriptor is then a large
    contiguous run per partition (8 KiB loads / 32 KiB stores).
  - 4 load DMAs per h-block (one per quadrant q = 2j+r) on the SWDGE
    (gpsimd) ring — measured substantially faster than the HWDGE rings
    for HBM reads here — with bufs=4 so the ring streams continuously.
  - 4 DVE copies interleave w (stride-2 free-dim writes) and place the
    j row offset, entirely within partitions.
  - 1 store DMA per h-block on the ACT (scalar) HWDGE ring, fully
    overlapped with the loads; each partition writes output rows
    2h0..2h0+2*HT-1 of its (b, c) back to back (contiguous).
Ring separation matters: store waits must not head-of-line-block loads.
"""

import numpy as np

import concourse.bass as bass
import concourse.mybir as mybir
from concourse.tile import TileContext
from concourse.bass_utils import run_bass_kernel_spmd

N_CORES = 8
B, D, H, W = 16, 256, 128, 128
BS = 2                      # block size
C = D // (BS * BS)          # 64 output channels
B_SH = B // N_CORES         # 2 batches per core
HT = 16                     # input rows per tile block
FP32 = mybir.dt.float32

_cache = {}


def _split_multi_waits(nc, cap=1):
    """Hoist excess semaphore waits onto NoOps placed just before the
    instruction (same engine => same sequencer, same order).  The TPB
    instruction encodings have a single sync-wait slot; Tile sometimes
    attaches 2+ waits, which walrus rejects ("Too many sync wait
    commands")."""
    for fn in nc.m.functions:
        for blk in fn.blocks:
            insts = blk.instructions
            i = 0
            while i < len(insts):
                inst = insts[i]
                si = inst.sync_info
                if (
                    si is not None
                    and si.on_wait is not None
                    and len(si.on_wait) > cap
                    and inst.opcode not in ("AllEngineBarrier",)
                ):
                    extra = list(si.on_wait[:-cap])
                    keep = list(si.on_wait[-cap:])
                    for w in extra:
                        nop = mybir.InstNoOp(
                            name=nc.get_next_instruction_name(),
                            engine=inst.engine,
                            ins=[],
                            outs=[],
                            bass_nofuse=True,
                            sync_info=mybir.SyncInfo(on_wait=[w], on_update=[]),
                        )
                        insts.insert(i, nop)
                        i += 1
                    si.on_wait = keep
                    inst.sync_info = si
                i += 1


def _build(repeat=1):
    nc = bass.Bass()
    x = nc.declare_dram_parameter("input", [B_SH, D, H, W], FP32, isOutput=False)
    y = nc.declare_dram_parameter(
        "output", [B_SH, C, H * BS, W * BS], FP32, isOutput=True
    )

    with TileContext(nc) as tc:
        with (
            tc.tile_pool(name="in_p", bufs=4) as in_pool,
            tc.tile_pool(name="out_p", bufs=2) as out_pool,
        ):
            for h0 in [hh for _ in range(repeat) for hh in range(0, H, HT)]:
                # ---- load: 4 quadrants, partition=(b,c), HT rows each ----
                in_ts = []
                for q in range(4):
                    in_t = in_pool.tile([B_SH * C, HT * W], FP32, tag=f"in{q}")
                    src = x[:, q * C : (q + 1) * C, h0 : h0 + HT, :]
                    nc.gpsimd.dma_start(
                        out=in_t[:, :],
                        in_=src.rearrange("b c h w -> b c (h w)"),
                    )
                    in_ts.append(in_t)
                # ---- interleave into output layout (within partitions) ----
                # out_t free layout: [hl*512 + j*256 + 2w + r], i.e. output
                # rows 2(h0+hl) and 2(h0+hl)+1 back to back per hl.
                out_t = out_pool.tile([B_SH * C, HT * 2 * W * BS], FP32)
                out_v = out_t.rearrange(
                    "p (hl j w r) -> p hl j w r", hl=HT, j=2, w=W, r=2
                )
                for q in range(4):
                    j, r = divmod(q, 2)
                    nc.vector.tensor_copy(
                        out=out_v[:, :, j, :, r],  # [128, HT, W] stride-2
                        in_=in_ts[q].rearrange("p (hl w) -> p hl w", w=W),
                    )
                # ---- store: one DMA, 2*HT*1024 B contiguous / partition ----
                dst = y[:, :, BS * h0 : BS * (h0 + HT), :]
                nc.scalar.dma_start(
                    out=dst.rearrange("b c h w -> b c (h w)"),
                    in_=out_t[:, :],
                )
    _split_multi_waits(nc)
    return nc


def kernel(input: np.ndarray) -> np.ndarray:
    assert input.shape == (B, D, H, W), input.shape
    if "nc" not in _cache:
        _cache["nc"] = _build()
    nc = _cache["nc"]

    in_maps = [
        {"input": np.ascontiguousarray(input[k * B_SH : (k + 1) * B_SH])}
        for k in range(N_CORES)
    ]
    res = run_bass_kernel_spmd(nc, in_maps, list(range(N_CORES)))
    out = np.concatenate(
        [np.asarray(res.results[k]["output"]) for k in range(N_CORES)], axis=0
    )
    return out.astype(input.dtype, copy=False)
